# revision 10
# baseline (speedup 1.0000x reference)
"""AdaptiveSoftmax log-prob kernel for 8 TRN2 NeuronCores (Bass/Tile).

Problem: nn_AdaptiveSoftmax (VOCAB=50257, CUTOFF=(10000,50000,50257), D=1024,
B,T = 2,1024).  reference computes, for xi=[N=2048,1024], tgt=[N]:
  head_lp = log_softmax(xi @ head_w.T)                    # [N, 10002]
  out[:, :10002] = head_lp
  tail0_lp = log_softmax((xi@t0_a.T)@t0_b.T) + head_lp[:, 9999:10000]
  out[:, 10000:50000] = where(10000<=tgt<50000, tail0_lp, out[:, 10000:50000])
  tail1_lp = log_softmax((xi@t1_a.T)@t1_b.T) + head_lp[:, 10000:10001]
  out[:, 50000:50257] = where(tgt>=50000, tail1_lp, 0)

Sharding: vocab-parallel.  Each core owns 1/8 of the head rows (padded
10016=8*1252), 1/8 of t0_b rows (5000), and 1/8 of padded tail1 rows
(264=8*33).  Every core processes all 2048 tokens.  The log-sum-exp
normalizers are combined with one small AllReduce per token-block, which
also carries the owner-gated head logits 9999..10001 (priors + the
head/tail boundary columns).  Compute in bf16 (weights pre-transposed and
cast on host), f32 accumulation.  Logits are tiny (|l| < ~7) so max-free
logsumexp is numerically safe.

All 8 cores run one SPMD graph; per-core differences enter via input data
(weight shards, flag vectors).
"""

import numpy as np
import ml_dtypes

import concourse.bass as bass
import concourse.mybir as mybir
import concourse.tile as tile
from concourse import bacc
from concourse.bass_utils import run_bass_kernel_spmd

BF16 = mybir.dt.bfloat16
F32 = mybir.dt.float32
AF = mybir.ActivationFunctionType
ALU = mybir.AluOpType

P = 128
D = 1024
KT = D // P          # 8 k-tiles over the model dim
NTOK = 2048
NCORES = 8

VOCAB = 50257
CUT0, CUT1 = 10000, 50000
HEAD_REAL = 10002            # 10000 words + 2 cluster logits
HEAD_W = 1252                # per-core head cols (padded total 10016)
HEAD_PAD = HEAD_W * NCORES - HEAD_REAL       # 14 zero columns (on core 7)
T0_W = 5000                  # per-core tail0 cols (40000 total, exact)
T1_REAL = VOCAB - CUT1       # 257
T1_W = 33                    # per-core tail1 cols (padded total 264)
T1_PAD = T1_W * NCORES - T1_REAL             # 7 zero columns (on core 7)
OUT_W = HEAD_W + T0_W + T1_W                 # 6285 per-core output cols

BLOCK = 512                  # max tokens per pipeline block
NBLK = NTOK // BLOCK
TPB = BLOCK // P             # max tok-tiles per block
BLOCKS = [4, 4, 4, 4]     # tile counts per pipeline block (sum=16)

# chunk table: (out_col_offset, width, kind, weight_col_offset)
# kind: 0=head (lhsT=xT, k=8), 1=tail0 (lhsT=projT, k=8), 2=tail1 (lhsT=proj1T, k=2)
CHUNKS = []
for off in range(0, HEAD_W, 512):
    CHUNKS.append((off, min(512, HEAD_W - off), 0, off))
for off in range(0, T0_W, 512):
    CHUNKS.append((HEAD_W + off, min(512, T0_W - off), 1, off))
CHUNKS.append((HEAD_W + T0_W, T1_W, 2, 0))
NCH = len(CHUNKS)            # 14
HEAD_SLOTS = [i for i, c in enumerate(CHUNKS) if c[2] == 0]
T0_SLOTS = [i for i, c in enumerate(CHUNKS) if c[2] == 1]
T1_SLOT = [i for i, c in enumerate(CHUNKS) if c[2] == 2][0]
# head cols 9999,10000,10001 live on core 7 at slice cols 1235..1237
E_OFF = 9999 - 7 * HEAD_W    # 1235

_BUILT = None  # cached compiled graph across kernel() calls


def build_nc(repeat=1, use_collective=True, psum_bufs=6, wbufs=4,
             outbufs=4, dedup_proj=False, loop_n=None, blocks=None,
             epi_lag=True):
    """Build the SPMD graph.  repeat>1 unrolls the whole kernel N times
    (for slope timing); loop_n wraps the kernel in a hardware For_i loop
    (timing only — forces the collective stub since collectives are not
    allowed inside control flow)."""
    if loop_n is not None:
        use_collective = False
    if blocks is None:
        blocks = BLOCKS
    assert sum(blocks) == NTOK // P, blocks

    nc = bacc.Bacc("TRN2", target_bir_lowering=False, debug=False,
                   num_devices=NCORES)

    xT = nc.dram_tensor("xT", [D, NTOK], BF16, kind="ExternalInput")
    hwT = nc.dram_tensor("hwT", [D, HEAD_W], BF16, kind="ExternalInput")
    t0bT = nc.dram_tensor("t0bT", [D, T0_W], BF16, kind="ExternalInput")
    t1bT = nc.dram_tensor("t1bT", [D // 4, T1_W], BF16, kind="ExternalInput")
    t0aT = nc.dram_tensor("t0aT", [D, P if dedup_proj else D], BF16,
                          kind="ExternalInput")
    t1aT = nc.dram_tensor("t1aT", [D, D // 4], BF16, kind="ExternalInput")
    masks = nc.dram_tensor("masks", [P, NTOK // P, 2], F32, kind="ExternalInput")
    flags = nc.dram_tensor("flags", [P, 2], F32, kind="ExternalInput")
    out = nc.dram_tensor("out", [NTOK, OUT_W], F32, kind="ExternalOutput")

    xT_r = xT.rearrange("(k p) t -> p k t", p=P)
    hwT_r = hwT.rearrange("(k p) n -> p k n", p=P)
    t0bT_r = t0bT.rearrange("(k p) n -> p k n", p=P)
    t1bT_r = t1bT.rearrange("(k p) n -> p k n", p=P)
    t0aT_r = t0aT.rearrange("(k p) n -> p k n", p=P)
    t1aT_r = t1aT.rearrange("(k p) n -> p k n", p=P)

    rg = [list(range(NCORES))]

    with tile.TileContext(nc) as tc:
        with (
            tc.tile_pool(name="const", bufs=1) as const,
            tc.tile_pool(name="xpool", bufs=2) as xpool,
            tc.tile_pool(name="projpool", bufs=2) as projpool,
            tc.tile_pool(name="wpool", bufs=wbufs) as wpool,
            tc.tile_pool(name="stpool", bufs=2) as stpool,
            tc.tile_pool(name="accpool", bufs=2) as accpool,
            tc.tile_pool(name="scpool", bufs=2) as scpool,
            tc.tile_pool(name="outpool", bufs=outbufs) as outpool,
            tc.tile_pool(name="psum", bufs=psum_bufs, space="PSUM") as psum,
            tc.tile_pool(name="dram", bufs=2, space="DRAM") as dram,
        ):
            # ---- constants ----
            t0aT_sb = const.tile([P, KT, P if dedup_proj else D], BF16)
            nc.sync.dma_start(t0aT_sb[:], t0aT_r)
            t1aT_sb = const.tile([P, KT, D // 4], BF16)
            nc.sync.dma_start(t1aT_sb[:], t1aT_r)
            t1bT_sb = const.tile([P, 2, T1_W], BF16)
            nc.sync.dma_start(t1bT_sb[:], t1bT_r)
            masks_sb = const.tile([P, NTOK // P, 2], F32)
            nc.sync.dma_start(masks_sb[:], masks[:])
            flags_sb = const.tile([P, 2], F32)
            nc.sync.dma_start(flags_sb[:], flags[:])
            zero_b = const.tile([P, 1], F32)
            nc.vector.memset(zero_b[:], 0.0)
            hpad_b = const.tile([P, 1], F32)
            nc.vector.memset(hpad_b[:], -float(HEAD_PAD))
            t1pad_b = const.tile([P, 1], F32)
            nc.vector.memset(t1pad_b[:], -float(T1_PAD))

            def emit_stats(tile0, tpb):
                tok0 = tile0 * P
                nblk = tpb * P

                # ---- activations for this token block ----
                xT_sb = xpool.tile([P, KT, nblk], BF16, tag="xT")
                nc.sync.dma_start(xT_sb[:], xT_r[:, :, tok0:tok0 + nblk])

                # projT(b) = t0_a @ x(b).T  -> [D, BLOCK] bf16
                projT_sb = projpool.tile([P, KT, nblk], BF16, tag="projT")
                if dedup_proj:
                    # each core computes only its 128 d'-rows (its t0aT
                    # column slice arrives via input data); AG reassembles
                    pslice = projpool.tile([P, nblk], BF16, tag="pslice")
                    for tc_ in range(0, nblk, 512):
                        tw = min(512, nblk - tc_)
                        ps = psum.tile([P, 512], F32, tag="mm")
                        for k in range(KT):
                            nc.tensor.matmul(
                                ps[:, :tw], t0aT_sb[:, k, 0:P],
                                xT_sb[:, k, tc_:tc_ + tw],
                                start=(k == 0), stop=(k == KT - 1))
                        nc.vector.tensor_copy(pslice[:, tc_:tc_ + tw], ps[:, :tw])
                    ag_in = dram.tile([P, BLOCK], BF16, tag="ag_in")
                    ag_out = dram.tile([D, BLOCK], BF16, tag="ag_out",
                                       addr_space="Shared")
                    nc.sync.dma_start(ag_in[:, :nblk], pslice[:])
                    if use_collective:
                        nc.gpsimd.collective_compute(
                            "AllGather", ALU.bypass, replica_groups=rg,
                            ins=[ag_in[:]], outs=[ag_out[:]])
                    else:
                        nc.sync.dma_start(
                            ag_out.rearrange("(kd p) t -> p kd t", p=P)[:, 0],
                            ag_in[:])
                    nc.sync.dma_start(
                        projT_sb[:],
                        ag_out.rearrange("(kd p) t -> p kd t", p=P)[:, :, :nblk])
                else:
                    for kd in range(KT):
                        for tc_ in range(0, nblk, 512):
                            tw = min(512, nblk - tc_)
                            ps = psum.tile([P, 512], F32, tag="mm")
                            for k in range(KT):
                                nc.tensor.matmul(
                                    ps[:, :tw], t0aT_sb[:, k, kd * P:(kd + 1) * P],
                                    xT_sb[:, k, tc_:tc_ + tw],
                                    start=(k == 0), stop=(k == KT - 1))
                            nc.vector.tensor_copy(projT_sb[:, kd, tc_:tc_ + tw],
                                                  ps[:, :tw])

                # proj1T(b) = t1_a @ x(b).T -> [256, nblk] bf16
                proj1T_sb = projpool.tile([P, 2, nblk], BF16, tag="proj1T")
                for kd in range(2):
                    for tc_ in range(0, nblk, 512):
                        tw = min(512, nblk - tc_)
                        ps = psum.tile([P, 512], F32, tag="mm")
                        for k in range(KT):
                            nc.tensor.matmul(
                                ps[:, :tw], t1aT_sb[:, k, kd * P:(kd + 1) * P],
                                xT_sb[:, k, tc_:tc_ + tw],
                                start=(k == 0), stop=(k == KT - 1))
                        nc.vector.tensor_copy(proj1T_sb[:, kd, tc_:tc_ + tw],
                                              ps[:, :tw])

                # ---- logit chunks: matmul -> staged bf16 logits + exp sums ----
                staged = stpool.tile([P, tpb, OUT_W], BF16, tag="staged",
                                     padded_shape=[P, TPB, OUT_W])
                acc = accpool.tile([P, tpb, NCH], F32, tag="acc",
                                   padded_shape=[P, TPB, NCH])
                for ci, (off, w, kind, woff) in enumerate(CHUNKS):
                    if kind == 0:
                        w_sb = wpool.tile([P, KT, 512], BF16, tag="w")
                        nc.sync.dma_start(w_sb[:, :, :w], hwT_r[:, :, woff:woff + w])
                        lhs, nk = xT_sb, KT
                    elif kind == 1:
                        w_sb = wpool.tile([P, KT, 512], BF16, tag="w")
                        nc.sync.dma_start(w_sb[:, :, :w], t0bT_r[:, :, woff:woff + w])
                        lhs, nk = projT_sb, KT
                    else:
                        w_sb, lhs, nk = t1bT_sb, proj1T_sb, 2
                    for i in range(tpb):
                        ps = psum.tile([P, 512], F32, tag="mm")
                        for k in range(nk):
                            nc.tensor.matmul(
                                ps[:, :w], lhs[:, k, i * P:(i + 1) * P],
                                w_sb[:, k, :w], start=(k == 0), stop=(k == nk - 1))
                        nc.vector.tensor_copy(staged[:, i, off:off + w], ps[:, :w])
                        dead = psum.tile([P, 512], F32, tag="dead", bufs=1)
                        nc.scalar.activation(
                            dead[:, :w], staged[:, i, off:off + w], AF.Exp,
                            bias=zero_b[:], accum_out=acc[:, i, ci:ci + 1])

                # ---- build AllReduce payload ----
                # layout [P, TPB, 6]: (s_t0, s_head, s_t1, l9999, l10000, l10001)
                # the l-slots are owner-gated LOGITS (linear, so add-AR works)
                ar_sb = accpool.tile([P, tpb, 6], F32, tag="ar_sb",
                                     padded_shape=[P, TPB, 6])
                h0, h1 = HEAD_SLOTS[0], HEAD_SLOTS[-1] + 1
                t0a_, t0b_ = T0_SLOTS[0], T0_SLOTS[-1] + 1
                for i in range(tpb):
                    nc.vector.reduce_sum(ar_sb[:, i, 0:1], acc[:, i, t0a_:t0b_],
                                         axis=mybir.AxisListType.X)
                    nc.vector.reduce_sum(ar_sb[:, i, 1:2], acc[:, i, h0:h1],
                                         axis=mybir.AxisListType.X)
                    nc.vector.tensor_copy(ar_sb[:, i, 2:3],
                                          acc[:, i, T1_SLOT:T1_SLOT + 1])
                    nc.vector.tensor_scalar_mul(ar_sb[:, i, 3:6],
                                                staged[:, i, E_OFF:E_OFF + 3],
                                                flags_sb[:, 0:1])

                ar_in = dram.tile([P, tpb, 6], F32, tag="ar_in",
                                  padded_shape=[P, TPB, 6])
                ar_out = dram.tile([P, tpb, 6], F32, tag="ar_out",
                                   padded_shape=[P, TPB, 6],
                                   addr_space="Shared")
                nc.sync.dma_start(ar_in[:], ar_sb[:])
                if use_collective:
                    nc.gpsimd.collective_compute(
                        "AllReduce", ALU.add, replica_groups=rg,
                        ins=[ar_in[:]], outs=[ar_out[:]])
                else:
                    nc.sync.dma_start(ar_out[:], ar_in[:])
                arr = accpool.tile([P, tpb, 6], F32, tag="arr",
                                   padded_shape=[P, TPB, 6])
                nc.sync.dma_start(arr[:], ar_out[:])
                return staged, arr

            def emit_epilogue(tile0, tpb, staged, arr):
                tok0 = tile0 * P
                # ---- post-AR per-token scalars ----
                # lns: (lse_t0, lse_h, lse_t1); arr[...,3:6] = (l9999,l10000,l10001)
                lns = scpool.tile([P, tpb, 3], F32, tag="lns",
                                  padded_shape=[P, TPB, 3])
                nc.scalar.activation(lns[:, :, 0:1], arr[:, :, 0:1], AF.Ln,
                                     bias=zero_b[:])
                nc.scalar.activation(lns[:, :, 1:2], arr[:, :, 1:2], AF.Ln,
                                     bias=hpad_b[:])
                nc.scalar.activation(lns[:, :, 2:3], arr[:, :, 2:3], AF.Ln,
                                     bias=t1pad_b[:])

                ch = scpool.tile([P, tpb], F32, tag="ch",
                                 padded_shape=[P, TPB])       # -lse_h
                c0 = scpool.tile([P, tpb], F32, tag="c0",
                                 padded_shape=[P, TPB])       # l9999-lse_h-lse_t0
                c1 = scpool.tile([P, tpb], F32, tag="c1",
                                 padded_shape=[P, TPB])       # l10000-lse_h-lse_t1
                bfix = scpool.tile([P, tpb, 2], F32, tag="bfix",
                                   padded_shape=[P, TPB, 2])
                tmp = scpool.tile([P, tpb, 2], F32, tag="tmp",
                                  padded_shape=[P, TPB, 2])
                m0_blk = masks_sb[:, tile0:tile0 + tpb, 0]
                nc.vector.tensor_scalar_mul(ch[:], lns[:, :, 1], -1.0)
                nc.vector.tensor_sub(tmp[:, :, 0], arr[:, :, 3], lns[:, :, 1])
                nc.vector.tensor_sub(c0[:], tmp[:, :, 0], lns[:, :, 0])
                nc.vector.tensor_sub(tmp[:, :, 0], arr[:, :, 4], lns[:, :, 1])
                nc.vector.tensor_sub(c1[:], tmp[:, :, 0], lns[:, :, 2])
                # boundary fixup cols 10000/10001 (core0 only, unmasked rows):
                # bfix_j = (1-m0) * core0flag * (l_{10000+j} - lse_h)
                nc.vector.tensor_sub(tmp[:, :, 0], arr[:, :, 4], lns[:, :, 1])
                nc.vector.tensor_sub(tmp[:, :, 1], arr[:, :, 5], lns[:, :, 1])
                u = scpool.tile([P, tpb], F32, tag="u",
                                padded_shape=[P, TPB])
                nc.vector.tensor_scalar(u[:], m0_blk, -1.0, 1.0,
                                        ALU.mult, ALU.add)
                for j in range(2):
                    nc.vector.tensor_mul(bfix[:, :, j], tmp[:, :, j], u[:])
                    nc.vector.tensor_scalar_mul(bfix[:, :, j], bfix[:, :, j],
                                                flags_sb[:, 1:2])

                # ---- epilogue: out = (staged + c) * m, DMA out ----
                for ci, (off, w, kind, woff) in enumerate(CHUNKS):
                    for i in range(tpb):
                        st = staged[:, i, off:off + w]
                        o = outpool.tile([P, 512], F32, tag="o")
                        if kind == 0:
                            nc.vector.tensor_scalar_add(o[:, :w], st,
                                                        ch[:, i:i + 1])
                        elif kind == 1:
                            nc.vector.tensor_scalar(
                                o[:, :w], st, c0[:, i:i + 1],
                                masks_sb[:, tile0 + i, 0:1],
                                ALU.add, ALU.mult)
                            if woff == 0:
                                nc.vector.tensor_add(o[:, 0:2], o[:, 0:2],
                                                     bfix[:, i, 0:2])
                        else:
                            nc.vector.tensor_scalar(
                                o[:, :w], st, c1[:, i:i + 1],
                                masks_sb[:, tile0 + i, 1:2],
                                ALU.add, ALU.mult)
                        nc.sync.dma_start(
                            out[tok0 + i * P:tok0 + (i + 1) * P, off:off + w],
                            o[:, :w])

            def emit_all():
                for rep in range(repeat):
                    pend = None
                    tile0 = 0
                    for tpb in blocks:
                        staged, arr = emit_stats(tile0, tpb)
                        if not epi_lag and pend is None:
                            emit_epilogue(tile0, tpb, staged, arr)
                        else:
                            if pend is not None:
                                emit_epilogue(*pend)
                            pend = (tile0, tpb, staged, arr)
                        tile0 += tpb
                    if pend is not None:
                        emit_epilogue(*pend)

            if loop_n is not None:
                with tc.For_i(0, loop_n, 1):
                    emit_all()
            else:
                emit_all()

    nc.compile()
    return nc


def _to_bf16(a):
    return np.ascontiguousarray(a).astype(ml_dtypes.bfloat16)


def make_in_maps(input, target, head_w, t0_a, t0_b, t1_a, t1_b,
                 dedup_proj=False):
    x = np.asarray(input, dtype=np.float32).reshape(NTOK, D)
    tgt = np.asarray(target).reshape(NTOK)
    head_w = np.asarray(head_w, dtype=np.float32)
    t0_a = np.asarray(t0_a, dtype=np.float32)
    t0_b = np.asarray(t0_b, dtype=np.float32)
    t1_a = np.asarray(t1_a, dtype=np.float32)
    t1_b = np.asarray(t1_b, dtype=np.float32)

    xT = _to_bf16(x.T)                                    # [D, NTOK]
    hw_pad = np.zeros((HEAD_W * NCORES, D), np.float32)
    hw_pad[:HEAD_REAL] = head_w
    hwT = _to_bf16(hw_pad.T)                              # [D, 10016]
    t0bT = _to_bf16(t0_b.T)                               # [D, 40000]
    t1_pad = np.zeros((T1_W * NCORES, D // 4), np.float32)
    t1_pad[:T1_REAL] = t1_b
    t1bT = _to_bf16(t1_pad.T)                             # [256, 264]
    t0aT = _to_bf16(t0_a.T)
    t1aT = _to_bf16(t1_a.T)

    m0 = ((tgt >= CUT0) & (tgt < CUT1)).astype(np.float32)
    m1 = (tgt >= CUT1).astype(np.float32)
    masks = np.stack([m0.reshape(NTOK // P, P).T,
                      m1.reshape(NTOK // P, P).T], axis=-1)  # [P, 16, 2]
    masks = np.ascontiguousarray(masks, dtype=np.float32)

    in_maps = []
    for c in range(NCORES):
        flags = np.zeros((P, 2), np.float32)
        flags[:, 0] = 1.0 if c == NCORES - 1 else 0.0
        flags[:, 1] = 1.0 if c == 0 else 0.0
        in_maps.append({
            "xT": xT,
            "hwT": np.ascontiguousarray(hwT[:, c * HEAD_W:(c + 1) * HEAD_W]),
            "t0bT": np.ascontiguousarray(t0bT[:, c * T0_W:(c + 1) * T0_W]),
            "t1bT": np.ascontiguousarray(t1bT[:, c * T1_W:(c + 1) * T1_W]),
            "t0aT": (np.ascontiguousarray(t0aT[:, c * P:(c + 1) * P])
                     if dedup_proj else t0aT),
            "t1aT": t1aT,
            "masks": masks,
            "flags": flags,
        })
    return in_maps


def assemble(results):
    """results: list of per-core dicts with 'out' [NTOK, OUT_W] f32."""
    full = np.zeros((NTOK, VOCAB), np.float32)
    for c in range(NCORES):
        o = results[c]["out"]
        # head slice: global cols [1252c, 1252(c+1)) clipped to < 10000
        lo = c * HEAD_W
        hi = min((c + 1) * HEAD_W, CUT0)
        if hi > lo:
            full[:, lo:hi] = o[:, :hi - lo]
        # tail0 slice: global cols [10000+5000c, 10000+5000(c+1))
        full[:, CUT0 + c * T0_W:CUT0 + (c + 1) * T0_W] = o[:, HEAD_W:HEAD_W + T0_W]
        # tail1 slice: global cols [50000+33c, ...) clipped to vocab
        lo1 = CUT1 + c * T1_W
        hi1 = min(lo1 + T1_W, VOCAB)
        if hi1 > lo1:
            full[:, lo1:hi1] = o[:, HEAD_W + T0_W:HEAD_W + T0_W + hi1 - lo1]
    return full.reshape(2, NTOK // 2, VOCAB)


def kernel(input, target, head_w, t0_a, t0_b, t1_a, t1_b):
    global _BUILT
    if _BUILT is None:
        _BUILT = build_nc()
    nc = _BUILT
    in_maps = make_in_maps(input, target, head_w, t0_a, t0_b, t1_a, t1_b)
    res = run_bass_kernel_spmd(nc, in_maps, core_ids=list(range(NCORES)))
    return assemble(res.results)


if __name__ == "__main__":
    import time
    t0 = time.time()
    nc = build_nc()
    print(f"build+compile: {time.time() - t0:.1f}s")


# revision 12
# speedup vs baseline: 183.8300x; 183.8300x over previous
"""AdaptiveSoftmax log-prob kernel for 8 TRN2 NeuronCores (Bass/Tile).

Problem: nn_AdaptiveSoftmax (VOCAB=50257, CUTOFF=(10000,50000,50257), D=1024,
B,T = 2,1024).  reference computes, for xi=[N=2048,1024], tgt=[N]:
  head_lp = log_softmax(xi @ head_w.T)                    # [N, 10002]
  out[:, :10002] = head_lp
  tail0_lp = log_softmax((xi@t0_a.T)@t0_b.T) + head_lp[:, 9999:10000]
  out[:, 10000:50000] = where(10000<=tgt<50000, tail0_lp, out[:, 10000:50000])
  tail1_lp = log_softmax((xi@t1_a.T)@t1_b.T) + head_lp[:, 10000:10001]
  out[:, 50000:50257] = where(tgt>=50000, tail1_lp, 0)

Sharding: vocab-parallel.  Each core owns 1/8 of the head rows (padded
10016=8*1252), 1/8 of t0_b rows (5000), and 1/8 of padded tail1 rows
(264=8*33).  Every core processes all 2048 tokens.  The log-sum-exp
normalizers are combined with one small AllReduce per token-block, which
also carries the owner-gated head logits 9999..10001 (priors + the
head/tail boundary columns).  Compute in bf16 (weights pre-transposed and
cast on host), f32 accumulation.  Logits are tiny (|l| < ~7) so max-free
logsumexp is numerically safe.

All 8 cores run one SPMD graph; per-core differences enter via input data
(weight shards, flag vectors).
"""

import numpy as np
import ml_dtypes

import concourse.bass as bass
import concourse.mybir as mybir
import concourse.tile as tile
from concourse import bacc
from concourse.bass_utils import run_bass_kernel_spmd

BF16 = mybir.dt.bfloat16
F32 = mybir.dt.float32
AF = mybir.ActivationFunctionType
ALU = mybir.AluOpType

P = 128
D = 1024
KT = D // P          # 8 k-tiles over the model dim
NTOK = 2048
NCORES = 8

VOCAB = 50257
CUT0, CUT1 = 10000, 50000
HEAD_REAL = 10002            # 10000 words + 2 cluster logits
HEAD_W = 1252                # per-core head cols (padded total 10016)
HEAD_PAD = HEAD_W * NCORES - HEAD_REAL       # 14 zero columns (on core 7)
T0_W = 5000                  # per-core tail0 cols (40000 total, exact)
T1_REAL = VOCAB - CUT1       # 257
T1_W = 33                    # per-core tail1 cols (padded total 264)
T1_PAD = T1_W * NCORES - T1_REAL             # 7 zero columns (on core 7)
OUT_W = HEAD_W + T0_W + T1_W                 # 6285 per-core output cols

BLOCK = 512                  # max tokens per pipeline block
NBLK = NTOK // BLOCK
TPB = BLOCK // P             # max tok-tiles per block
BLOCKS = [4, 4, 4, 4]     # tile counts per pipeline block (sum=16)

# chunk table: (out_col_offset, width, kind, weight_col_offset)
# kind: 0=head (lhsT=xT, k=8), 1=tail0 (lhsT=projT, k=8), 2=tail1 (lhsT=proj1T, k=2)
CHUNKS = []
for off in range(0, HEAD_W, 512):
    CHUNKS.append((off, min(512, HEAD_W - off), 0, off))
for off in range(0, T0_W, 512):
    CHUNKS.append((HEAD_W + off, min(512, T0_W - off), 1, off))
CHUNKS.append((HEAD_W + T0_W, T1_W, 2, 0))
NCH = len(CHUNKS)            # 14
HEAD_SLOTS = [i for i, c in enumerate(CHUNKS) if c[2] == 0]
T0_SLOTS = [i for i, c in enumerate(CHUNKS) if c[2] == 1]
T1_SLOT = [i for i, c in enumerate(CHUNKS) if c[2] == 2][0]
# head cols 9999,10000,10001 live on core 7 at slice cols 1235..1237
E_OFF = 9999 - 7 * HEAD_W    # 1235

_BUILT = None  # cached compiled graph across kernel() calls


def build_nc(repeat=1, use_collective=True, psum_bufs=6, wbufs=4,
             outbufs=6, dedup_proj=False, loop_n=None, blocks=None,
             epi_lag=True, copy_split=0):
    """Build the SPMD graph.  repeat>1 unrolls the whole kernel N times
    (for slope timing); loop_n wraps the kernel in a hardware For_i loop
    (timing only — forces the collective stub since collectives are not
    allowed inside control flow)."""
    if loop_n is not None:
        use_collective = False
    if blocks is None:
        blocks = BLOCKS
    assert sum(blocks) == NTOK // P, blocks

    nc = bacc.Bacc("TRN2", target_bir_lowering=False, debug=False,
                   num_devices=NCORES)

    xT = nc.dram_tensor("xT", [D, NTOK], BF16, kind="ExternalInput")
    hwT = nc.dram_tensor("hwT", [D, HEAD_W], BF16, kind="ExternalInput")
    t0bT = nc.dram_tensor("t0bT", [D, T0_W], BF16, kind="ExternalInput")
    t1bT = nc.dram_tensor("t1bT", [D // 4, T1_W], BF16, kind="ExternalInput")
    t0aT = nc.dram_tensor("t0aT", [D, P if dedup_proj else D], BF16,
                          kind="ExternalInput")
    t1aT = nc.dram_tensor("t1aT", [D, D // 4], BF16, kind="ExternalInput")
    masks = nc.dram_tensor("masks", [P, NTOK // P, 2], F32, kind="ExternalInput")
    flags = nc.dram_tensor("flags", [P, 2], F32, kind="ExternalInput")
    out = nc.dram_tensor("out", [NTOK, OUT_W], F32, kind="ExternalOutput")

    xT_r = xT.rearrange("(k p) t -> p k t", p=P)
    hwT_r = hwT.rearrange("(k p) n -> p k n", p=P)
    t0bT_r = t0bT.rearrange("(k p) n -> p k n", p=P)
    t1bT_r = t1bT.rearrange("(k p) n -> p k n", p=P)
    t0aT_r = t0aT.rearrange("(k p) n -> p k n", p=P)
    t1aT_r = t1aT.rearrange("(k p) n -> p k n", p=P)

    rg = [list(range(NCORES))]

    with tile.TileContext(nc) as tc:
        with (
            tc.tile_pool(name="const", bufs=1) as const,
            tc.tile_pool(name="xpool", bufs=2) as xpool,
            tc.tile_pool(name="projpool", bufs=2) as projpool,
            tc.tile_pool(name="wpool", bufs=wbufs) as wpool,
            tc.tile_pool(name="stpool", bufs=2) as stpool,
            tc.tile_pool(name="accpool", bufs=2) as accpool,
            tc.tile_pool(name="scpool", bufs=2) as scpool,
            tc.tile_pool(name="outpool", bufs=outbufs) as outpool,
            tc.tile_pool(name="psum", bufs=psum_bufs, space="PSUM") as psum,
            tc.tile_pool(name="dram", bufs=2, space="DRAM") as dram,
        ):
            # ---- constants ----
            t0aT_sb = const.tile([P, KT, P if dedup_proj else D], BF16)
            nc.sync.dma_start(t0aT_sb[:], t0aT_r)
            t1aT_sb = const.tile([P, KT, D // 4], BF16)
            nc.sync.dma_start(t1aT_sb[:], t1aT_r)
            t1bT_sb = const.tile([P, 2, T1_W], BF16)
            nc.sync.dma_start(t1bT_sb[:], t1bT_r)
            masks_sb = const.tile([P, NTOK // P, 2], F32)
            nc.sync.dma_start(masks_sb[:], masks[:])
            flags_sb = const.tile([P, 2], F32)
            nc.sync.dma_start(flags_sb[:], flags[:])
            zero_b = const.tile([P, 1], F32)
            nc.vector.memset(zero_b[:], 0.0)
            hpad_b = const.tile([P, 1], F32)
            nc.vector.memset(hpad_b[:], -float(HEAD_PAD))
            t1pad_b = const.tile([P, 1], F32)
            nc.vector.memset(t1pad_b[:], -float(T1_PAD))

            def emit_stats(tile0, tpb):
                tok0 = tile0 * P
                nblk = tpb * P

                # ---- activations for this token block ----
                xT_sb = xpool.tile([P, KT, nblk], BF16, tag="xT")
                nc.sync.dma_start(xT_sb[:], xT_r[:, :, tok0:tok0 + nblk])

                # projT(b) = t0_a @ x(b).T  -> [D, BLOCK] bf16
                projT_sb = projpool.tile([P, KT, nblk], BF16, tag="projT")
                if dedup_proj:
                    # each core computes only its 128 d'-rows (its t0aT
                    # column slice arrives via input data); AG reassembles
                    pslice = projpool.tile([P, nblk], BF16, tag="pslice")
                    for tc_ in range(0, nblk, 512):
                        tw = min(512, nblk - tc_)
                        ps = psum.tile([P, 512], F32, tag="mm")
                        for k in range(KT):
                            nc.tensor.matmul(
                                ps[:, :tw], t0aT_sb[:, k, 0:P],
                                xT_sb[:, k, tc_:tc_ + tw],
                                start=(k == 0), stop=(k == KT - 1))
                        nc.vector.tensor_copy(pslice[:, tc_:tc_ + tw], ps[:, :tw])
                    ag_in = dram.tile([P, BLOCK], BF16, tag="ag_in")
                    ag_out = dram.tile([D, BLOCK], BF16, tag="ag_out",
                                       addr_space="Shared")
                    nc.sync.dma_start(ag_in[:, :nblk], pslice[:])
                    if use_collective:
                        nc.gpsimd.collective_compute(
                            "AllGather", ALU.bypass, replica_groups=rg,
                            ins=[ag_in[:]], outs=[ag_out[:]])
                    else:
                        nc.sync.dma_start(
                            ag_out.rearrange("(kd p) t -> p kd t", p=P)[:, 0],
                            ag_in[:])
                    nc.sync.dma_start(
                        projT_sb[:],
                        ag_out.rearrange("(kd p) t -> p kd t", p=P)[:, :, :nblk])
                else:
                    for kd in range(KT):
                        for tc_ in range(0, nblk, 512):
                            tw = min(512, nblk - tc_)
                            ps = psum.tile([P, 512], F32, tag="mm")
                            for k in range(KT):
                                nc.tensor.matmul(
                                    ps[:, :tw], t0aT_sb[:, k, kd * P:(kd + 1) * P],
                                    xT_sb[:, k, tc_:tc_ + tw],
                                    start=(k == 0), stop=(k == KT - 1))
                            nc.vector.tensor_copy(projT_sb[:, kd, tc_:tc_ + tw],
                                                  ps[:, :tw])

                # proj1T(b) = t1_a @ x(b).T -> [256, nblk] bf16
                proj1T_sb = projpool.tile([P, 2, nblk], BF16, tag="proj1T")
                for kd in range(2):
                    for tc_ in range(0, nblk, 512):
                        tw = min(512, nblk - tc_)
                        ps = psum.tile([P, 512], F32, tag="mm")
                        for k in range(KT):
                            nc.tensor.matmul(
                                ps[:, :tw], t1aT_sb[:, k, kd * P:(kd + 1) * P],
                                xT_sb[:, k, tc_:tc_ + tw],
                                start=(k == 0), stop=(k == KT - 1))
                        nc.vector.tensor_copy(proj1T_sb[:, kd, tc_:tc_ + tw],
                                              ps[:, :tw])

                # ---- logit chunks: matmul -> staged bf16 logits + exp sums ----
                staged = stpool.tile([P, tpb, OUT_W], BF16, tag="staged",
                                     padded_shape=[P, TPB, OUT_W])
                acc = accpool.tile([P, tpb, NCH], F32, tag="acc",
                                   padded_shape=[P, TPB, NCH])
                for ci, (off, w, kind, woff) in enumerate(CHUNKS):
                    if kind == 0:
                        w_sb = wpool.tile([P, KT, 512], BF16, tag="w")
                        nc.sync.dma_start(w_sb[:, :, :w], hwT_r[:, :, woff:woff + w])
                        lhs, nk = xT_sb, KT
                    elif kind == 1:
                        w_sb = wpool.tile([P, KT, 512], BF16, tag="w")
                        nc.sync.dma_start(w_sb[:, :, :w], t0bT_r[:, :, woff:woff + w])
                        lhs, nk = projT_sb, KT
                    else:
                        w_sb, lhs, nk = t1bT_sb, proj1T_sb, 2
                    for i in range(tpb):
                        ps = psum.tile([P, 512], F32, tag="mm")
                        for k in range(nk):
                            nc.tensor.matmul(
                                ps[:, :w], lhs[:, k, i * P:(i + 1) * P],
                                w_sb[:, k, :w], start=(k == 0), stop=(k == nk - 1))
                        if copy_split and (ci % copy_split == 0):
                            nc.scalar.copy(staged[:, i, off:off + w], ps[:, :w])
                        else:
                            nc.vector.tensor_copy(staged[:, i, off:off + w],
                                                  ps[:, :w])
                        dead = psum.tile([P, 512], F32, tag="dead", bufs=1)
                        nc.scalar.activation(
                            dead[:, :w], staged[:, i, off:off + w], AF.Exp,
                            bias=zero_b[:], accum_out=acc[:, i, ci:ci + 1])

                # ---- build AllReduce payload ----
                # layout [P, TPB, 6]: (s_t0, s_head, s_t1, l9999, l10000, l10001)
                # the l-slots are owner-gated LOGITS (linear, so add-AR works)
                ar_sb = accpool.tile([P, tpb, 6], F32, tag="ar_sb",
                                     padded_shape=[P, TPB, 6])
                h0, h1 = HEAD_SLOTS[0], HEAD_SLOTS[-1] + 1
                t0a_, t0b_ = T0_SLOTS[0], T0_SLOTS[-1] + 1
                for i in range(tpb):
                    nc.vector.reduce_sum(ar_sb[:, i, 0:1], acc[:, i, t0a_:t0b_],
                                         axis=mybir.AxisListType.X)
                    nc.vector.reduce_sum(ar_sb[:, i, 1:2], acc[:, i, h0:h1],
                                         axis=mybir.AxisListType.X)
                    nc.vector.tensor_copy(ar_sb[:, i, 2:3],
                                          acc[:, i, T1_SLOT:T1_SLOT + 1])
                    nc.vector.tensor_scalar_mul(ar_sb[:, i, 3:6],
                                                staged[:, i, E_OFF:E_OFF + 3],
                                                flags_sb[:, 0:1])

                ar_in = dram.tile([P, tpb, 6], F32, tag="ar_in",
                                  padded_shape=[P, TPB, 6])
                ar_out = dram.tile([P, tpb, 6], F32, tag="ar_out",
                                   padded_shape=[P, TPB, 6],
                                   addr_space="Shared")
                nc.sync.dma_start(ar_in[:], ar_sb[:])
                if use_collective:
                    nc.gpsimd.collective_compute(
                        "AllReduce", ALU.add, replica_groups=rg,
                        ins=[ar_in[:]], outs=[ar_out[:]])
                else:
                    nc.sync.dma_start(ar_out[:], ar_in[:])
                arr = accpool.tile([P, tpb, 6], F32, tag="arr",
                                   padded_shape=[P, TPB, 6])
                nc.sync.dma_start(arr[:], ar_out[:])
                return staged, arr

            def emit_epilogue(tile0, tpb, staged, arr):
                tok0 = tile0 * P
                # ---- post-AR per-token scalars ----
                # lns: (lse_t0, lse_h, lse_t1); arr[...,3:6] = (l9999,l10000,l10001)
                lns = scpool.tile([P, tpb, 3], F32, tag="lns",
                                  padded_shape=[P, TPB, 3])
                nc.scalar.activation(lns[:, :, 0:1], arr[:, :, 0:1], AF.Ln,
                                     bias=zero_b[:])
                nc.scalar.activation(lns[:, :, 1:2], arr[:, :, 1:2], AF.Ln,
                                     bias=hpad_b[:])
                nc.scalar.activation(lns[:, :, 2:3], arr[:, :, 2:3], AF.Ln,
                                     bias=t1pad_b[:])

                ch = scpool.tile([P, tpb], F32, tag="ch",
                                 padded_shape=[P, TPB])       # -lse_h
                c0 = scpool.tile([P, tpb], F32, tag="c0",
                                 padded_shape=[P, TPB])       # l9999-lse_h-lse_t0
                c1 = scpool.tile([P, tpb], F32, tag="c1",
                                 padded_shape=[P, TPB])       # l10000-lse_h-lse_t1
                bfix = scpool.tile([P, tpb, 2], F32, tag="bfix",
                                   padded_shape=[P, TPB, 2])
                tmp = scpool.tile([P, tpb, 2], F32, tag="tmp",
                                  padded_shape=[P, TPB, 2])
                m0_blk = masks_sb[:, tile0:tile0 + tpb, 0]
                nc.vector.tensor_scalar_mul(ch[:], lns[:, :, 1], -1.0)
                nc.vector.tensor_sub(tmp[:, :, 0], arr[:, :, 3], lns[:, :, 1])
                nc.vector.tensor_sub(c0[:], tmp[:, :, 0], lns[:, :, 0])
                nc.vector.tensor_sub(tmp[:, :, 0], arr[:, :, 4], lns[:, :, 1])
                nc.vector.tensor_sub(c1[:], tmp[:, :, 0], lns[:, :, 2])
                # boundary fixup cols 10000/10001 (core0 only, unmasked rows):
                # bfix_j = (1-m0) * core0flag * (l_{10000+j} - lse_h)
                nc.vector.tensor_sub(tmp[:, :, 0], arr[:, :, 4], lns[:, :, 1])
                nc.vector.tensor_sub(tmp[:, :, 1], arr[:, :, 5], lns[:, :, 1])
                u = scpool.tile([P, tpb], F32, tag="u",
                                padded_shape=[P, TPB])
                nc.vector.tensor_scalar(u[:], m0_blk, -1.0, 1.0,
                                        ALU.mult, ALU.add)
                for j in range(2):
                    nc.vector.tensor_mul(bfix[:, :, j], tmp[:, :, j], u[:])
                    nc.vector.tensor_scalar_mul(bfix[:, :, j], bfix[:, :, j],
                                                flags_sb[:, 1:2])

                # ---- epilogue: out = (staged + c) * m, DMA out ----
                for ci, (off, w, kind, woff) in enumerate(CHUNKS):
                    for i in range(tpb):
                        st = staged[:, i, off:off + w]
                        o = outpool.tile([P, 512], F32, tag="o")
                        if kind == 0:
                            nc.vector.tensor_scalar_add(o[:, :w], st,
                                                        ch[:, i:i + 1])
                        elif kind == 1:
                            nc.vector.tensor_scalar(
                                o[:, :w], st, c0[:, i:i + 1],
                                masks_sb[:, tile0 + i, 0:1],
                                ALU.add, ALU.mult)
                            if woff == 0:
                                nc.vector.tensor_add(o[:, 0:2], o[:, 0:2],
                                                     bfix[:, i, 0:2])
                        else:
                            nc.vector.tensor_scalar(
                                o[:, :w], st, c1[:, i:i + 1],
                                masks_sb[:, tile0 + i, 1:2],
                                ALU.add, ALU.mult)
                        nc.sync.dma_start(
                            out[tok0 + i * P:tok0 + (i + 1) * P, off:off + w],
                            o[:, :w])

            def emit_all():
                for rep in range(repeat):
                    pend = None
                    tile0 = 0
                    for tpb in blocks:
                        staged, arr = emit_stats(tile0, tpb)
                        if not epi_lag and pend is None:
                            emit_epilogue(tile0, tpb, staged, arr)
                        else:
                            if pend is not None:
                                emit_epilogue(*pend)
                            pend = (tile0, tpb, staged, arr)
                        tile0 += tpb
                    if pend is not None:
                        emit_epilogue(*pend)

            if loop_n is not None:
                with tc.For_i(0, loop_n, 1):
                    emit_all()
            else:
                emit_all()

    nc.compile()
    return nc


def _to_bf16(a):
    return np.ascontiguousarray(a).astype(ml_dtypes.bfloat16)


def make_in_maps(input, target, head_w, t0_a, t0_b, t1_a, t1_b,
                 dedup_proj=False):
    x = np.asarray(input, dtype=np.float32).reshape(NTOK, D)
    tgt = np.asarray(target).reshape(NTOK)
    head_w = np.asarray(head_w, dtype=np.float32)
    t0_a = np.asarray(t0_a, dtype=np.float32)
    t0_b = np.asarray(t0_b, dtype=np.float32)
    t1_a = np.asarray(t1_a, dtype=np.float32)
    t1_b = np.asarray(t1_b, dtype=np.float32)

    xT = _to_bf16(x.T)                                    # [D, NTOK]
    hw_pad = np.zeros((HEAD_W * NCORES, D), np.float32)
    hw_pad[:HEAD_REAL] = head_w
    hwT = _to_bf16(hw_pad.T)                              # [D, 10016]
    t0bT = _to_bf16(t0_b.T)                               # [D, 40000]
    t1_pad = np.zeros((T1_W * NCORES, D // 4), np.float32)
    t1_pad[:T1_REAL] = t1_b
    t1bT = _to_bf16(t1_pad.T)                             # [256, 264]
    t0aT = _to_bf16(t0_a.T)
    t1aT = _to_bf16(t1_a.T)

    m0 = ((tgt >= CUT0) & (tgt < CUT1)).astype(np.float32)
    m1 = (tgt >= CUT1).astype(np.float32)
    masks = np.stack([m0.reshape(NTOK // P, P).T,
                      m1.reshape(NTOK // P, P).T], axis=-1)  # [P, 16, 2]
    masks = np.ascontiguousarray(masks, dtype=np.float32)

    in_maps = []
    for c in range(NCORES):
        flags = np.zeros((P, 2), np.float32)
        flags[:, 0] = 1.0 if c == NCORES - 1 else 0.0
        flags[:, 1] = 1.0 if c == 0 else 0.0
        in_maps.append({
            "xT": xT,
            "hwT": np.ascontiguousarray(hwT[:, c * HEAD_W:(c + 1) * HEAD_W]),
            "t0bT": np.ascontiguousarray(t0bT[:, c * T0_W:(c + 1) * T0_W]),
            "t1bT": np.ascontiguousarray(t1bT[:, c * T1_W:(c + 1) * T1_W]),
            "t0aT": (np.ascontiguousarray(t0aT[:, c * P:(c + 1) * P])
                     if dedup_proj else t0aT),
            "t1aT": t1aT,
            "masks": masks,
            "flags": flags,
        })
    return in_maps


def assemble(results):
    """results: list of per-core dicts with 'out' [NTOK, OUT_W] f32."""
    full = np.zeros((NTOK, VOCAB), np.float32)
    for c in range(NCORES):
        o = results[c]["out"]
        # head slice: global cols [1252c, 1252(c+1)) clipped to < 10000
        lo = c * HEAD_W
        hi = min((c + 1) * HEAD_W, CUT0)
        if hi > lo:
            full[:, lo:hi] = o[:, :hi - lo]
        # tail0 slice: global cols [10000+5000c, 10000+5000(c+1))
        full[:, CUT0 + c * T0_W:CUT0 + (c + 1) * T0_W] = o[:, HEAD_W:HEAD_W + T0_W]
        # tail1 slice: global cols [50000+33c, ...) clipped to vocab
        lo1 = CUT1 + c * T1_W
        hi1 = min(lo1 + T1_W, VOCAB)
        if hi1 > lo1:
            full[:, lo1:hi1] = o[:, HEAD_W + T0_W:HEAD_W + T0_W + hi1 - lo1]
    return full.reshape(2, NTOK // 2, VOCAB)


def kernel(input, target, head_w, t0_a, t0_b, t1_a, t1_b):
    global _BUILT
    if _BUILT is None:
        _BUILT = build_nc(dedup_proj=True)
    nc = _BUILT
    in_maps = make_in_maps(input, target, head_w, t0_a, t0_b, t1_a, t1_b,
                           dedup_proj=True)
    res = run_bass_kernel_spmd(nc, in_maps, core_ids=list(range(NCORES)))
    return assemble(res.results)


if __name__ == "__main__":
    import time
    t0 = time.time()
    nc = build_nc()
    print(f"build+compile: {time.time() - t0:.1f}s")
